# revision 60
# baseline (speedup 1.0000x reference)
import sys, os
for p in ("/opt/trn_rl_repo", "/root/.axon_site/_ro/trn_rl_repo"):
    if os.path.isdir(p) and p not in sys.path:
        sys.path.insert(0, p)

import numpy as np
import ml_dtypes

import concourse.bass as bass
import concourse.bacc as bacc
import concourse.tile as tile
from concourse import mybir
from concourse.bass_utils import run_bass_kernel_spmd

BF16 = ml_dtypes.bfloat16

# Problem constants (hardcoded per contract)
B, S, D = 2, 2048, 2048
HEADS, HD, NKV = 32, 64, 8
NCORES = 8
TPG = 4             # tensor-parallel groups per batch
HPC = HEADS // TPG  # 8 q-heads per core
KVPC = NKV // TPG   # 2 kv heads per core
NP = 4              # head pairs per core (kv0-head, kv1-head)
ST = S // 128       # 16 s-tiles
DT = D // 128       # 16 d_in-chunks
NPH = 4             # s-phases of 512
EPS = 1e-6

f32 = mybir.dt.float32
bf16 = mybir.dt.bfloat16

_prog = None


def _build_program():
    nc = bacc.Bacc("TRN2", target_bir_lowering=False, debug=False)

    # activations/weights arrive pre-transposed to partition-major layout so
    # every DMA line is contiguous
    xT_d = nc.dram_tensor("xT", [128, DT * S], bf16, kind="ExternalInput").ap()
    wqkv_d = nc.dram_tensor("wqkv", [128, DT * 768], bf16, kind="ExternalInput").ap()
    wo_d = nc.dram_tensor("wo", [128, NP * D], bf16, kind="ExternalInput").ap()
    cos_d = nc.dram_tensor("cosT", [128, S], bf16, kind="ExternalInput").ap()
    sin_d = nc.dram_tensor("sinT2", [128, S], bf16, kind="ExternalInput").ap()
    mdiag_d = nc.dram_tensor("mdiagT", [128, 128], bf16, kind="ExternalInput").ap()
    eind_d = nc.dram_tensor("eind", [128, 2], bf16, kind="ExternalInput").ap()
    ebq_d = nc.dram_tensor("ebq", [2, 128], bf16, kind="ExternalInput").ap()
    ebk_d = nc.dram_tensor("ebk", [2, 128], bf16, kind="ExternalInput").ap()
    psw_d = nc.dram_tensor("pswap", [128, 128], bf16, kind="ExternalInput").ap()
    id_d = nc.dram_tensor("id128", [128, 128], bf16, kind="ExternalInput").ap()
    out_d = nc.dram_tensor("out", [S, D], f32, kind="ExternalOutput").ap()

    with tile.TileContext(nc) as tc:
        with (
            tc.tile_pool(name="big", bufs=1) as big,
            tc.tile_pool(name="raw", bufs=5) as rawp,
            tc.tile_pool(name="sq", bufs=2) as sqp,
            tc.tile_pool(name="rn", bufs=5) as rnp,
            tc.tile_pool(name="rms", bufs=2) as rmsp,
            tc.tile_pool(name="t1", bufs=2) as t1p,
            tc.tile_pool(name="u", bufs=10) as up,
            tc.tile_pool(name="vt", bufs=2) as vtp,
            tc.tile_pool(name="pt", bufs=4) as ptp,
            tc.tile_pool(name="ys", bufs=4) as ysp,
            tc.tile_pool(name="xw", bufs=2) as xwp,
            tc.tile_pool(name="oc", bufs=2) as ocp,
            tc.tile_pool(name="slab", bufs=2, space="PSUM") as slabp,
            tc.tile_pool(name="ot", bufs=2, space="PSUM") as otp,
            tc.tile_pool(name="gp", bufs=2, space="PSUM") as gpp,
        ):
            # ---- resident SBUF tensors (small constants first) ----
            eind_sb = big.tile([128, 2], bf16)
            nc.sync.dma_start(out=eind_sb, in_=eind_d)
            ebq_sb = big.tile([2, 128], bf16)
            nc.sync.dma_start(out=ebq_sb, in_=ebq_d)
            ebk_sb = big.tile([2, 128], bf16)
            nc.sync.dma_start(out=ebk_sb, in_=ebk_d)
            psw_sb = big.tile([128, 128], bf16)
            nc.sync.dma_start(out=psw_sb, in_=psw_d)
            id_sb = big.tile([128, 128], bf16)
            nc.sync.dma_start(out=id_sb, in_=id_d)
            mdiagT_sb = big.tile([128, 128], bf16)
            nc.sync.dma_start(out=mdiagT_sb, in_=mdiag_d)
            eps_sb = big.tile([2, 1], f32)
            nc.vector.memset(eps_sb, EPS)

            # weights + activations split per d-chunk so compute starts early;
            # xT streamed per 512-wide s-window (double buffered) to save SBUF
            wqkv_sb = big.tile([128, DT, 768], bf16)

            def load_xw(ph):
                xw = xwp.tile([128, DT, 512], bf16, tag="xw")
                for d in range(DT):
                    nc.sync.dma_start(out=xw[:, d, :],
                                      in_=xT_d[:, d * S + ph * 512:d * S + (ph + 1) * 512])
                return xw

            xw_cur = xwp.tile([128, DT, 512], bf16, tag="xw")
            for d in range(DT):
                # K/V weight columns first: the first proj chunks (c=4,5) read them
                nc.sync.dma_start(out=wqkv_sb[:, d, 512:768],
                                  in_=wqkv_d[:, d * 768 + 512:(d + 1) * 768])
                nc.sync.dma_start(out=xw_cur[:, d, :], in_=xT_d[:, d * S:d * S + 512])
            for d in range(DT):
                nc.sync.dma_start(out=wqkv_sb[:, d, 0:512],
                                  in_=wqkv_d[:, d * 768:d * 768 + 512])
            cos_sb = big.tile([128, S], bf16)
            nc.sync.dma_start(out=cos_sb, in_=cos_d)
            sin_sb = big.tile([128, S], bf16)
            nc.sync.dma_start(out=sin_sb, in_=sin_d)
            wo_sb = big.tile([128, NP, D], bf16)
            nc.sync.dma_start(out=wo_sb, in_=wo_d)

            QT_sb = big.tile([128, NP, S], bf16)   # pair p: parts 0:64 head p, 64:128 head p+4
            KT_sb = big.tile([128, S], bf16)       # parts 0:64 kv0 dims, 64:128 kv1 dims
            # per s-tile: [64 v0 | 64 ones | 64 v1 | 64 ones]; the ones columns
            # make the AV matmul replicate the softmax denom across 64 psum rows
            Vbuf = big.tile([128, ST, 256], bf16)
            nc.vector.memset(Vbuf, 1.0)
            OT_sb = big.tile([128, NP, S], bf16)   # normalized O^T

            def stage3(cq):
                # output projection for q-tiles of chunk cq
                for i in range(4 * cq, 4 * cq + 4):
                    for ns in range(4):
                        yp = gpp.tile([128, 512], f32, tag="gp")
                        for t in range(NP):
                            nc.tensor.matmul(yp[:], OT_sb[:, t, i * 128:(i + 1) * 128],
                                             wo_sb[:, t, ns * 512:(ns + 1) * 512],
                                             start=(t == 0), stop=(t == NP - 1))
                        ys = ysp.tile([128, 512], f32, tag="ys")
                        nc.vector.tensor_copy(ys[:], yp[:])
                        nc.sync.dma_start(out=out_d[i * 128:(i + 1) * 128,
                                                    ns * 512:(ns + 1) * 512], in_=ys[:])

            # ---- stage 1 (projections + rms prep) as resumable chunk units ----
            def make_stage1(xw, ph):
                sc = slice(ph * 512, (ph + 1) * 512)
                st = dict(raws={}, rns={}, tms={}, us={}, sqs={}, pending=[], sc=sc)

                st["ssqs"] = {}

                def emit_ssq():
                    # ssq (PE) + stage to SBUF (DVE); the ACT-table-using sqrt
                    # is deferred to the phase junction so it can't thrash the
                    # Exp table inside the attention stream
                    c = st["pending"].pop(0)
                    ssq = gpp.tile([2, 512], f32, tag="gp")
                    nc.tensor.matmul(ssq[:], eind_sb[:], st["sqs"][c][:],
                                     start=True, stop=True)
                    sb = rmsp.tile([2, 512], f32, tag="ssq", bufs=5)
                    nc.vector.tensor_copy(sb[:], ssq[:])
                    st["ssqs"][c] = sb

                st["emit_ssq"] = emit_ssq

                def chunk(c):
                    pj = gpp.tile([128, 512], f32, tag="gp")
                    for d in range(DT):
                        nc.tensor.matmul(pj[:], wqkv_sb[:, d, c * 128:(c + 1) * 128],
                                         xw[:, d, :], start=(d == 0), stop=(d == DT - 1))
                    # ssq lags two chunks behind so the PE never waits on the
                    # Vector queue's sq here
                    if len(st["pending"]) >= 2:
                        emit_ssq()
                    if c == 5:
                        vt = vtp.tile([128, 512], bf16, tag="vt")
                        nc.vector.tensor_copy(vt[:], pj[:])
                        for t_ in range(4):
                            tp_ = gpp.tile([128, 128], bf16, tag="gp")
                            nc.tensor.transpose(tp_[:], vt[:, t_ * 128:(t_ + 1) * 128],
                                                id_sb[:])
                            nc.vector.tensor_copy(Vbuf[:, ph * 4 + t_, 0:64], tp_[:, 0:64])
                            nc.vector.tensor_copy(Vbuf[:, ph * 4 + t_, 128:192],
                                                  tp_[:, 64:128])
                    else:
                        r = rawp.tile([128, 512], f32, tag="raw")
                        nc.vector.tensor_copy(r[:], pj[:])
                        st["raws"][c] = r
                        sq = sqp.tile([128, 512], bf16, tag="sq")
                        nc.vector.tensor_mul(sq[:], r[:], r[:])
                        st["sqs"][c] = sq
                        st["pending"].append(c)
                        # rope muls on gpsimd can start as soon as raw lands
                        tm = up.tile([128, 512], bf16, tag="u")
                        nc.gpsimd.tensor_mul(tm[:], r[:], cos_sb[:, sc])
                        u = up.tile([128, 512], bf16, tag="u")
                        nc.gpsimd.tensor_mul(u[:], r[:], sin_sb[:, sc])
                        st["tms"][c] = tm
                        st["us"][c] = u

                qchunks = [lambda c=c: chunk(c) for c in (4, 0, 1, 2, 3)]
                vchunk = lambda: chunk(5)
                return qchunks, vchunk, st

            def rope(st, pre_filler=None):
                sc = st["sc"]
                # finish the rms chains: ALL sqrts back-to-back (one Sqrt-table
                # swap per phase; Copy uses a different table set, so the casts
                # go to the Vector engine instead)
                while st["pending"]:
                    st["emit_ssq"]()
                rmss = {}
                for c in (4, 0, 1, 2, 3):
                    rms = rmsp.tile([2, 512], f32, tag="rms", bufs=5)
                    nc.scalar.activation(rms[:], st["ssqs"][c][:],
                                         mybir.ActivationFunctionType.Sqrt,
                                         bias=eps_sb[:], scale=1.0 / HD)
                    rmss[c] = rms
                for c in (4, 0, 1, 2, 3):
                    rnf = rmsp.tile([2, 512], f32, tag="rnf")
                    nc.vector.reciprocal_approx_fast(rnf[:], rmss[c][:])
                    rn = rnp.tile([2, 512], bf16, tag="rn")
                    with nc.allow_low_precision("bf16 cast of rmsnorm recip"):
                        nc.vector.tensor_copy(rn[:], rnf[:])
                    st["rns"][c] = rn
                # PE filler while the sqrt/recip/cast chains drain
                if pre_filler is not None:
                    pre_filler()
                # rope combine + rms scale (scale applied last)
                for c in (4, 0, 1, 2, 3):
                    # partition-swap u (rotate-half) via PE permutation matmul
                    u2 = gpp.tile([128, 512], f32, tag="gp")
                    nc.tensor.matmul(u2[:], psw_sb[:], st["us"][c][:],
                                     start=True, stop=True)
                    w = t1p.tile([128, 512], bf16, tag="t1")
                    nc.vector.tensor_add(w[:], st["tms"][c][:], u2[:])
                    bc = gpp.tile([128, 512], f32, tag="gp")
                    nc.tensor.matmul(bc[:], ebq_sb[:] if c < 4 else ebk_sb[:],
                                     st["rns"][c][:], start=True, stop=True)
                    tgt = QT_sb[:, c, sc] if c < 4 else KT_sb[:, sc]
                    nc.vector.tensor_mul(tgt, w[:], bc[:])

            def stage3_units(cq):
                units = []
                for i in range(4 * cq, 4 * cq + 4):
                    for ns in range(4):
                        def unit(i=i, ns=ns):
                            yp = gpp.tile([128, 512], f32, tag="gp")
                            for t in range(NP):
                                nc.tensor.matmul(yp[:],
                                                 OT_sb[:, t, i * 128:(i + 1) * 128],
                                                 wo_sb[:, t, ns * 512:(ns + 1) * 512],
                                                 start=(t == 0), stop=(t == NP - 1))
                            ys = ysp.tile([128, 512], f32, tag="ys")
                            nc.vector.tensor_copy(ys[:], yp[:])
                            nc.sync.dma_start(out=out_d[i * 128:(i + 1) * 128,
                                                        ns * 512:(ns + 1) * 512],
                                              in_=ys[:])
                        units.append(unit)
                return units

            # prologue: phase 0's projections + rope run standalone
            qchunks, vchunk, cur_st = make_stage1(xw_cur, 0)
            for f in qchunks:
                f()
            vchunk()
            rope(cur_st)

            for ph in range(NPH):
                sc = slice(ph * 512, (ph + 1) * 512)
                # PE filler units interleaved into this phase's attention:
                # next phase's q projections + prev phase's output projection
                fillers = []
                junction = []
                if ph + 1 < NPH:
                    xw_nxt = load_xw(ph + 1)
                    qchunks, vchunk, nxt_st = make_stage1(xw_nxt, ph + 1)
                    fillers += qchunks
                if ph >= 1:
                    s3u = stage3_units(ph - 1)
                    if ph + 1 < NPH:
                        fillers += s3u[:10]
                        junction += s3u[10:]
                    else:
                        fillers += s3u

                # ======== stage 2: attention for q-chunk ph ========
                cq = ph
                for p in range(NP):
                    otA = otp.tile([128, 512], f32, tag="ot")
                    otB = otp.tile([128, 512], f32, tag="ot")
                    prev = None
                    for jg in range(2 * cq + 2):
                        slA = slabp.tile([128, 1024], f32, tag="slab")
                        slB = slabp.tile([128, 1024], f32, tag="slab")
                        offs = [128 * max(0, 2 * jg + jj - 4 * cq) for jj in range(2)]
                        exps = []
                        for half, sl, kt0 in ((0, slA, 0), (1, slB, 64)):
                            for jj in range(2):
                                j = 2 * jg + jj
                                off = offs[jj]
                                diag = j >= 4 * cq
                                qs_ = QT_sb[:, p, cq * 512 + off:(cq + 1) * 512]
                                nc.tensor.matmul(sl[:, jj * 512 + off:(jj + 1) * 512],
                                                 KT_sb[kt0:kt0 + 64, j * 128:(j + 1) * 128],
                                                 qs_[kt0:kt0 + 64, :], start=True, stop=not diag,
                                                 skip_group_check=diag)
                                if diag:
                                    a0 = jj * 512 + off
                                    nc.tensor.matmul(sl[:, a0:a0 + 128], mdiagT_sb[:], id_sb[:],
                                                     start=False, stop=True, skip_group_check=True)
                            pt = ptp.tile([128, 1024], bf16, tag="pt")
                            if offs[0] == 0 and offs[1] == 0:
                                nc.scalar.activation(pt[:], sl[:],
                                                     mybir.ActivationFunctionType.Exp,
                                                     scale=0.125)
                            else:
                                for jj in range(2):
                                    a0, a1 = jj * 512 + offs[jj], (jj + 1) * 512
                                    nc.scalar.activation(pt[:, a0:a1], sl[:, a0:a1],
                                                         mybir.ActivationFunctionType.Exp,
                                                         scale=0.125)
                            exps.append(pt)
                        if prev is not None:
                            for half, ot in ((0, otA), (1, otB)):
                                for jj in range(2):
                                    j = 2 * prev[0] + jj
                                    off = 128 * max(0, j - 4 * cq)
                                    nc.tensor.matmul(ot[:, off:512],
                                                     Vbuf[:, j, 128 * half:128 * half + 128],
                                                     prev[1 + half][:, jj * 512 + off:(jj + 1) * 512],
                                                     start=(j == 0), stop=(j == 4 * cq + 3))
                        prev = (jg, exps[0], exps[1])
                    for half, ot in ((0, otA), (1, otB)):
                        for jj in range(2):
                            j = 2 * prev[0] + jj
                            off = 128 * max(0, j - 4 * cq)
                            nc.tensor.matmul(ot[:, off:512],
                                             Vbuf[:, j, 128 * half:128 * half + 128],
                                             prev[1 + half][:, jj * 512 + off:(jj + 1) * 512],
                                             start=(j == 0), stop=(j == 4 * cq + 3))
                    # ---- normalize: OT = ot[0:64] * recip(denom rows), all DVE ----
                    # (denom staged to a base-0 SBUF tile: the bit-trick
                    # reciprocal must read base-0 SBUF)
                    for ot, o0 in ((otA, 0), (otB, 64)):
                        dn_ = ocp.tile([64, 512], f32, tag="dn")
                        nc.vector.tensor_copy(dn_[:], ot[64:128, :])
                        rdn = rmsp.tile([64, 512], f32, tag="rd", bufs=2)
                        nc.vector.reciprocal_approx_fast(rdn[:], dn_[:])
                        nc.vector.tensor_mul(OT_sb[o0:o0 + 64, p, sc],
                                             ot[0:64, :], rdn[:])

                    # interleave PE filler work between pairs
                    k = (len(fillers) + NP - 1 - p) // (NP - p)
                    for f in fillers[:k]:
                        f()
                    fillers = fillers[k:]

                for f in fillers:
                    f()
                if ph + 1 < NPH:
                    rope(nxt_st, pre_filler=vchunk)
                    cur_st = nxt_st
                for f in junction:
                    f()

            stage3(NPH - 1)
    nc.compile()
    return nc


def _get_prog():
    global _prog
    if _prog is None:
        _prog = _build_program()
    return _prog


def _prep_inputs(x, mask, cos, sin, Wq, Wk, Wv, Wo, q_scale, k_scale):
    cos = np.asarray(cos, np.float32)
    sin = np.asarray(sin, np.float32)
    qs, ks = np.asarray(q_scale, np.float32), np.asarray(k_scale, np.float32)

    dup = lambda a: np.concatenate([a, a], axis=0).astype(BF16)      # [128, S]
    cosT = dup(cos.T)
    # sinT2[e] = sin[dst(e)] * sgn(dst(e)), dst(e) = partner dim of e
    sinT2 = dup(np.concatenate([sin[:, 32:], -sin[:, :32]], axis=1).T)

    k_ = np.arange(128)
    mdiagT = np.where(k_[:, None] < k_[None, :], -1e9, 0.0).astype(np.float32)
    eind = np.zeros((128, 2), np.float32)
    eind[0:64, 0] = 1.0
    eind[64:128, 1] = 1.0
    ebq = np.zeros((2, 128), np.float32)
    ebq[0, 0:64] = qs
    ebq[1, 64:128] = qs
    ebk = np.zeros((2, 128), np.float32)
    ebk[0, 0:64] = ks
    ebk[1, 64:128] = ks
    psw = np.zeros((128, 128), np.float32)
    for m in range(128):
        src = m + 32 if (m % 64) < 32 else m - 32
        psw[src, m] = 1.0

    # partition-major relayout: [T*128, N] -> [128, T*N] (contiguous DMA lines)
    def pmajor(a):
        t = a.shape[0] // 128
        return np.ascontiguousarray(
            a.reshape(t, 128, a.shape[1]).transpose(1, 0, 2).reshape(128, -1))

    in_maps = []
    xTs = [pmajor(np.ascontiguousarray(x[b].T).astype(BF16)) for b in range(B)]
    for c in range(NCORES):
        b, g = c // TPG, c % TPG
        kvs = slice(g * KVPC * HD, (g + 1) * KVPC * HD)
        cols = []
        for p in range(NP):
            cols.append(Wq[:, (g * HPC + p) * HD:(g * HPC + p + 1) * HD])
            cols.append(Wq[:, (g * HPC + p + 4) * HD:(g * HPC + p + 5) * HD])
        wqkv = pmajor(np.concatenate(cols + [Wk[:, kvs], Wv[:, kvs]], axis=1).astype(BF16))
        ORD = [0, 4, 1, 5, 2, 6, 3, 7]
        wo = pmajor(np.concatenate(
            [Wo[(g * HPC + o) * HD:(g * HPC + o + 1) * HD, :] for o in ORD],
            axis=0).astype(BF16))
        in_maps.append(dict(xT=xTs[b], wqkv=wqkv, wo=wo, cosT=cosT, sinT2=sinT2,
                            mdiagT=mdiagT.astype(BF16), eind=eind.astype(BF16),
                            ebq=ebq.astype(BF16), ebk=ebk.astype(BF16),
                            pswap=psw.astype(BF16),
                            id128=np.eye(128, dtype=np.float32).astype(BF16)))
    return in_maps


def kernel(x, mask, cos, sin, Wq, Wk, Wv, Wo, q_scale, k_scale, _trace=False):
    nc = _get_prog()
    in_maps = _prep_inputs(x, mask, cos, sin, Wq, Wk, Wv, Wo, q_scale, k_scale)
    res = run_bass_kernel_spmd(nc, in_maps, core_ids=list(range(NCORES)), trace=_trace)
    kernel.last_results = res
    out = np.zeros((B, S, D), np.float32)
    for c in range(NCORES):
        out[c // TPG] += res.results[c]["out"]
    return out
